# revision 4
# baseline (speedup 1.0000x reference)
"""Trainium2 Bass kernel for nn_Discrimitor (embedding_lookup two-tower MLP).

Strategy (8 NeuronCores, data-parallel over the batch):
  - Replicate the 1M x 100 f32 embedding table, host-cast to fp16 and pad
    rows to 128 elements (256B rows) -> per-core HBM gather granularity is
    one 256B row.
  - Each core handles 65536 index pairs. Rows are fetched with SWDGE
    indirect DMA (gather): 4096 rows per call, int32 indices resident in
    SBUF, landing batch-major ([128 partitions, 32 rows x 128 fp16]).
  - Per 512-batch compute tile: 8 PE transposes (fp16, via identity) flip
    a/c rows to embed-major, DVE/ACT copy PSUM->SBUF fp16, DVE forms a*c,
    3 accumulating fp16 matmuls (K=128, N=512) compute hidden @ W1 into
    PSUM [64,512], ACT applies relu+b1 -> fp16, one matmul with W2 gives
    logits [1,512], ACT/DVE adds b2 into an output staging row which is
    DMA'd back 4096 logits at a time.

The kernel() entry takes FULL unsharded inputs and returns the FULL
[524288, 1] f32 output.
"""

import sys

for _p in ("/opt/trn_rl_repo",):
    if _p not in sys.path:
        sys.path.insert(0, _p)

import numpy as np

import concourse.bacc as bacc
import concourse.tile as tile
from concourse import bass, mybir
from concourse.bass_utils import run_bass_kernel_spmd
from concourse.masks import make_identity

# ---- problem constants (hardcoded per contract) ----
DOC_SIZE = 1_000_000
EMBED = 100
DP = 128          # padded row length (fp16 -> 256B rows)
H = 64
BATCH = 524288
CORES = 8
BC = BATCH // CORES          # 65536 batch elements per core

# ---- kernel tunables ----
GROUPS = 16                  # gather groups per core
RPG = BC // GROUPS           # rows gathered per call (a and c separately)
JPG = RPG // 128             # rows per partition per gather call
TILE = 512                   # batch elements per compute tile
TPG = RPG // TILE            # compute tiles per gather group

F16 = mybir.dt.float16
F32 = mybir.dt.float32
I32 = mybir.dt.int32
AF = mybir.ActivationFunctionType


def build_nc(doc_rows=DOC_SIZE, bc=BC, groups=GROUPS):
    """Build the per-core Bass module. Parametrized so tests can build a
    small config for CoreSim."""
    rpg = bc // groups
    jpg = rpg // 128
    tpg = rpg // TILE
    assert rpg % TILE == 0 and TILE == 512

    nc = bacc.Bacc("TRN2", target_bir_lowering=False)

    tbl = nc.dram_tensor("tbl", [doc_rows, DP], F16, kind="ExternalInput")
    ia = nc.dram_tensor("ia", [128, jpg * groups], I32, kind="ExternalInput")
    ic = nc.dram_tensor("ic", [128, jpg * groups], I32, kind="ExternalInput")
    w1 = nc.dram_tensor("w1", [DP, 3 * H], F16, kind="ExternalInput")
    w2 = nc.dram_tensor("w2", [H, 1], F16, kind="ExternalInput")
    b1 = nc.dram_tensor("b1", [H, 1], F32, kind="ExternalInput")
    b2 = nc.dram_tensor("b2", [1, 1], F32, kind="ExternalInput")
    out = nc.dram_tensor("out", [bc], F32, kind="ExternalOutput")

    with tile.TileContext(nc) as tc:
        with (
            tc.tile_pool(name="singles", bufs=1) as singles,
            tc.tile_pool(name="graw", bufs=2) as graw,
            tc.tile_pool(name="tsb", bufs=3) as tsb,
            tc.tile_pool(name="h1sb", bufs=2) as h1sb,
            tc.tile_pool(name="stage", bufs=2) as stagep,
            tc.tile_pool(name="ps_t", bufs=4, space="PSUM") as ps_t,
            tc.tile_pool(name="ps_h1", bufs=2, space="PSUM") as ps_h1,
            tc.tile_pool(name="ps_lg", bufs=2, space="PSUM") as ps_lg,
        ):
            # constants / weights / indices -> SBUF once
            w1_sb = singles.tile([DP, 3 * H], F16)
            nc.sync.dma_start(out=w1_sb[:], in_=w1[:])
            w2_sb = singles.tile([H, 1], F16)
            nc.sync.dma_start(out=w2_sb[:], in_=w2[:])
            b1_sb = singles.tile([H, 1], F32)
            nc.sync.dma_start(out=b1_sb[:], in_=b1[:])
            b2_sb = singles.tile([1, 1], F32)
            nc.sync.dma_start(out=b2_sb[:], in_=b2[:])
            ident = singles.tile([128, 128], F16)
            make_identity(nc, ident[:])
            ia_sb = singles.tile([128, jpg * groups], I32)
            nc.sync.dma_start(out=ia_sb[:], in_=ia[:])
            ic_sb = singles.tile([128, jpg * groups], I32)
            nc.sync.dma_start(out=ic_sb[:], in_=ic[:])

            for g in range(groups):
                # HW indirect DMA consumes exactly one offset per partition
                # per call -> gather 128 rows (one [128,128] block) per call.
                a_raw = graw.tile([128, rpg], F16, tag="araw")
                for j in range(jpg):
                    nc.gpsimd.indirect_dma_start(
                        out=a_raw[:, j * DP : (j + 1) * DP],
                        out_offset=None,
                        in_=tbl[:],
                        in_offset=bass.IndirectOffsetOnAxis(
                            ap=ia_sb[:, g * jpg + j : g * jpg + j + 1], axis=0
                        ),
                    )
                c_raw = graw.tile([128, rpg], F16, tag="craw")
                for j in range(jpg):
                    nc.gpsimd.indirect_dma_start(
                        out=c_raw[:, j * DP : (j + 1) * DP],
                        out_offset=None,
                        in_=tbl[:],
                        in_offset=bass.IndirectOffsetOnAxis(
                            ap=ic_sb[:, g * jpg + j : g * jpg + j + 1], axis=0
                        ),
                    )

                stage = stagep.tile([1, rpg], F32)

                for tt in range(tpg):
                    aT = tsb.tile([128, TILE], F16, tag="aT")
                    cT = tsb.tile([128, TILE], F16, tag="cT")
                    for u in range(4):
                        k = (tt * 4 + u) * 128
                        tp_a = ps_t.tile([128, 128], F16, tag="pst")
                        nc.tensor.transpose(
                            tp_a[:], a_raw[:, k : k + 128], ident[:]
                        )
                        nc.vector.tensor_copy(
                            out=aT[:, u * 128 : (u + 1) * 128], in_=tp_a[:]
                        )
                        tp_c = ps_t.tile([128, 128], F16, tag="pst")
                        nc.tensor.transpose(
                            tp_c[:], c_raw[:, k : k + 128], ident[:]
                        )
                        nc.scalar.activation(
                            out=cT[:, u * 128 : (u + 1) * 128],
                            in_=tp_c[:],
                            func=AF.Copy,
                        )
                    acT = tsb.tile([128, TILE], F16, tag="acT")
                    nc.vector.tensor_mul(acT[:], aT[:], cT[:])

                    h1p = ps_h1.tile([H, TILE], F32, tag="h1p")
                    nc.tensor.matmul(
                        h1p[:], w1_sb[:, 0:H], aT[:], start=True, stop=False
                    )
                    nc.tensor.matmul(
                        h1p[:], w1_sb[:, H : 2 * H], cT[:], start=False, stop=False
                    )
                    nc.tensor.matmul(
                        h1p[:], w1_sb[:, 2 * H : 3 * H], acT[:],
                        start=False, stop=True,
                    )
                    h1s = h1sb.tile([H, TILE], F16, tag="h1s")
                    nc.scalar.activation(
                        out=h1s[:], in_=h1p[:], func=AF.Relu, bias=b1_sb[:],
                        scale=1.0,
                    )
                    lgp = ps_lg.tile([1, TILE], F32, tag="lgp")
                    nc.tensor.matmul(
                        lgp[:], w2_sb[:], h1s[:], start=True, stop=True
                    )
                    dst = stage[0:1, tt * TILE : (tt + 1) * TILE]
                    if tt % 2 == 0:
                        nc.scalar.activation(
                            out=dst, in_=lgp[:], func=AF.Identity, bias=b2_sb[:],
                            scale=1.0,
                        )
                    else:
                        nc.vector.tensor_add(
                            out=dst, in0=lgp[:],
                            in1=b2_sb[:].to_broadcast([1, TILE]),
                        )

                nc.sync.dma_start(
                    out=out[g * rpg : (g + 1) * rpg].rearrange("(o n) -> o n", o=1),
                    in_=stage[:],
                )

    nc.compile()
    return nc


def prep_inputs(anchor_h, candidate_h, doc_embed, W1, b1, W2, b2,
                bc=BC, groups=GROUPS, cores=CORES):
    """Host-side packing of full inputs into per-core in_maps."""
    jpg = bc // groups // 128

    tbl16 = np.zeros((doc_embed.shape[0], DP), np.float16)
    tbl16[:, :EMBED] = np.asarray(doc_embed, np.float32)

    # W1 rows: [a(100); c(100); ac(100)] -> padded chunks of 128
    W1 = np.asarray(W1, np.float32)
    w1p = np.zeros((DP, 3 * H), np.float16)
    w1p[:EMBED, 0:H] = W1[0:EMBED]
    w1p[:EMBED, H : 2 * H] = W1[EMBED : 2 * EMBED]
    w1p[:EMBED, 2 * H : 3 * H] = W1[2 * EMBED : 3 * EMBED]

    w2p = np.asarray(W2, np.float32).astype(np.float16).reshape(H, 1)
    b1p = np.asarray(b1, np.float32).reshape(H, 1)
    b2p = np.asarray(b2, np.float32).reshape(1, 1)

    a_all = np.asarray(anchor_h).astype(np.int32)
    c_all = np.asarray(candidate_h).astype(np.int32)

    in_maps = []
    for c in range(cores):
        sl = slice(c * bc, (c + 1) * bc)
        # layout[p, g*jpg + j] = idx[g*rpg + j*128 + p]
        ia = (
            a_all[sl].reshape(groups, jpg, 128).transpose(2, 0, 1)
            .reshape(128, groups * jpg).copy()
        )
        icx = (
            c_all[sl].reshape(groups, jpg, 128).transpose(2, 0, 1)
            .reshape(128, groups * jpg).copy()
        )
        in_maps.append({
            "tbl": tbl16, "ia": ia, "ic": icx,
            "w1": w1p, "w2": w2p, "b1": b1p, "b2": b2p,
        })
    return in_maps


_NC_CACHE = {}


def get_nc():
    if "nc" not in _NC_CACHE:
        _NC_CACHE["nc"] = build_nc()
    return _NC_CACHE["nc"]


def kernel(anchor_h, candidate_h, doc_embed, W1, b1, W2, b2):
    nc = get_nc()
    in_maps = prep_inputs(anchor_h, candidate_h, doc_embed, W1, b1, W2, b2)
    res = run_bass_kernel_spmd(nc, in_maps, core_ids=list(range(CORES)))
    outs = [res.results[c]["out"] for c in range(CORES)]
    return np.concatenate(outs).reshape(BATCH, 1).astype(np.float32)


# revision 8
# speedup vs baseline: 55.7240x; 55.7240x over previous
"""Trainium2 Bass kernel for nn_Discrimitor (embedding_lookup two-tower MLP).

Strategy (8 NeuronCores, data-parallel over the batch):
  - Replicate the 1M x 100 f32 embedding table, host-cast to fp16 and pad
    rows to 128 elements (256B rows) -> per-core HBM gather granularity is
    one 256B row.
  - Each core handles 65536 index pairs. Rows are fetched with SWDGE
    indirect DMA (gather): 4096 rows per call, int32 indices resident in
    SBUF, landing batch-major ([128 partitions, 32 rows x 128 fp16]).
  - Per 512-batch compute tile: 8 PE transposes (fp16, via identity) flip
    a/c rows to embed-major, DVE/ACT copy PSUM->SBUF fp16, DVE forms a*c,
    3 accumulating fp16 matmuls (K=128, N=512) compute hidden @ W1 into
    PSUM [64,512], ACT applies relu+b1 -> fp16, one matmul with W2 gives
    logits [1,512], ACT/DVE adds b2 into an output staging row which is
    DMA'd back 4096 logits at a time.

The kernel() entry takes FULL unsharded inputs and returns the FULL
[524288, 1] f32 output.
"""

import sys

for _p in ("/opt/trn_rl_repo",):
    if _p not in sys.path:
        sys.path.insert(0, _p)

import numpy as np

import concourse.bacc as bacc
import concourse.tile as tile
from concourse import bass, mybir
from concourse.bass_utils import run_bass_kernel_spmd
from concourse.masks import make_identity

# ---- problem constants (hardcoded per contract) ----
DOC_SIZE = 1_000_000
EMBED = 100
DP = 128          # padded row length (fp16 -> 256B rows)
H = 64
BATCH = 524288
CORES = 8
BC = BATCH // CORES          # 65536 batch elements per core

# ---- kernel tunables ----
GROUPS = 16                  # gather groups per core
RPG = BC // GROUPS           # rows gathered per call (a and c separately)
JPG = RPG // 128             # rows per partition per gather call
TILE = 512                   # batch elements per compute tile
TPG = RPG // TILE            # compute tiles per gather group

F16 = mybir.dt.float16
F32 = mybir.dt.float32
I32 = mybir.dt.int32
AF = mybir.ActivationFunctionType


def build_nc(doc_rows=DOC_SIZE, bc=BC, groups=GROUPS, reps=1):
    """Build the per-core Bass module. Parametrized so tests can build a
    small config for CoreSim; reps>1 wraps the body in a hardware loop for
    dispatch-noise-free timing."""
    rpg = bc // groups
    jpg = rpg // 128
    tpg = rpg // TILE
    assert rpg % TILE == 0 and TILE == 512

    nc = bacc.Bacc("TRN2", target_bir_lowering=False)

    tbl = nc.dram_tensor("tbl", [doc_rows, DP], F16, kind="ExternalInput")
    ia = nc.dram_tensor("ia", [128, jpg * groups], I32, kind="ExternalInput")
    ic = nc.dram_tensor("ic", [128, jpg * groups], I32, kind="ExternalInput")
    w1 = nc.dram_tensor("w1", [DP, 3 * H], F16, kind="ExternalInput")
    w2 = nc.dram_tensor("w2", [H, 1], F16, kind="ExternalInput")
    b1 = nc.dram_tensor("b1", [H, 1], F32, kind="ExternalInput")
    b2 = nc.dram_tensor("b2", [1, 1], F32, kind="ExternalInput")
    out = nc.dram_tensor("out", [bc], F32, kind="ExternalOutput")

    with tile.TileContext(nc) as tc:
        with (
            tc.tile_pool(name="singles", bufs=1) as singles,
            tc.tile_pool(name="graw", bufs=3) as graw,
            tc.tile_pool(name="tsb", bufs=4) as tsb,
            tc.tile_pool(name="h1sb", bufs=2) as h1sb,
            tc.tile_pool(name="stage", bufs=2) as stagep,
            tc.tile_pool(name="ps_t", bufs=4, space="PSUM") as ps_t,
            tc.tile_pool(name="ps_h1", bufs=2, space="PSUM") as ps_h1,
            tc.tile_pool(name="ps_lg", bufs=2, space="PSUM") as ps_lg,
        ):
            # constants / weights / indices -> SBUF once
            w1_sb = singles.tile([DP, 3 * H], F16)
            nc.sync.dma_start(out=w1_sb[:], in_=w1[:])
            w2_sb = singles.tile([H, 1], F16)
            nc.sync.dma_start(out=w2_sb[:], in_=w2[:])
            b1_sb = singles.tile([H, 1], F32)
            nc.sync.dma_start(out=b1_sb[:], in_=b1[:])
            b2_sb = singles.tile([1, 1], F32)
            nc.sync.dma_start(out=b2_sb[:], in_=b2[:])
            ident = singles.tile([128, 128], F16)
            make_identity(nc, ident[:])
            ia_sb = singles.tile([128, jpg * groups], I32)
            nc.sync.dma_start(out=ia_sb[:], in_=ia[:])
            ic_sb = singles.tile([128, jpg * groups], I32)
            nc.sync.dma_start(out=ic_sb[:], in_=ic[:])

            rep_cm = tc.For_i(0, reps) if reps > 1 else None
            if rep_cm is not None:
                rep_cm.__enter__()
            for g in range(groups):
                # HW indirect DMA consumes exactly one offset per partition
                # per call -> gather 128 rows (one [128,128] block) per call.
                a_raw = graw.tile([128, rpg], F16, tag="araw")
                for j in range(jpg):
                    nc.gpsimd.indirect_dma_start(
                        out=a_raw[:, j * DP : (j + 1) * DP],
                        out_offset=None,
                        in_=tbl[:],
                        in_offset=bass.IndirectOffsetOnAxis(
                            ap=ia_sb[:, g * jpg + j : g * jpg + j + 1], axis=0
                        ),
                    )
                c_raw = graw.tile([128, rpg], F16, tag="craw")
                for j in range(jpg):
                    nc.gpsimd.indirect_dma_start(
                        out=c_raw[:, j * DP : (j + 1) * DP],
                        out_offset=None,
                        in_=tbl[:],
                        in_offset=bass.IndirectOffsetOnAxis(
                            ap=ic_sb[:, g * jpg + j : g * jpg + j + 1], axis=0
                        ),
                    )

                stage = stagep.tile([1, rpg], F32)

                for tt in range(tpg):
                    aT = tsb.tile([128, TILE], F16, tag="aT")
                    cT = tsb.tile([128, TILE], F16, tag="cT")
                    for u in range(4):
                        k = (tt * 4 + u) * 128
                        tp_a = ps_t.tile([128, 128], F16, tag="pst")
                        nc.tensor.transpose(
                            tp_a[:], a_raw[:, k : k + 128], ident[:]
                        )
                        nc.vector.tensor_copy(
                            out=aT[:, u * 128 : (u + 1) * 128], in_=tp_a[:]
                        )
                        tp_c = ps_t.tile([128, 128], F16, tag="pst")
                        nc.tensor.transpose(
                            tp_c[:], c_raw[:, k : k + 128], ident[:]
                        )
                        nc.scalar.activation(
                            out=cT[:, u * 128 : (u + 1) * 128],
                            in_=tp_c[:],
                            func=AF.Copy,
                        )
                    acT = tsb.tile([128, TILE], F16, tag="acT")
                    nc.vector.tensor_mul(acT[:], aT[:], cT[:])

                    h1p = ps_h1.tile([H, TILE], F32, tag="h1p")
                    nc.tensor.matmul(
                        h1p[:], w1_sb[:, 0:H], aT[:], start=True, stop=False
                    )
                    nc.tensor.matmul(
                        h1p[:], w1_sb[:, H : 2 * H], cT[:], start=False, stop=False
                    )
                    nc.tensor.matmul(
                        h1p[:], w1_sb[:, 2 * H : 3 * H], acT[:],
                        start=False, stop=True,
                    )
                    h1s = h1sb.tile([H, TILE], F16, tag="h1s")
                    nc.scalar.activation(
                        out=h1s[:], in_=h1p[:], func=AF.Relu, bias=b1_sb[:],
                        scale=1.0,
                    )
                    lgp = ps_lg.tile([1, TILE], F32, tag="lgp")
                    nc.tensor.matmul(
                        lgp[:], w2_sb[:], h1s[:], start=True, stop=True
                    )
                    dst = stage[0:1, tt * TILE : (tt + 1) * TILE]
                    if tt % 2 == 0:
                        nc.scalar.activation(
                            out=dst, in_=lgp[:], func=AF.Identity, bias=b2_sb[:],
                            scale=1.0,
                        )
                    else:
                        nc.vector.tensor_add(
                            out=dst, in0=lgp[:],
                            in1=b2_sb[:].to_broadcast([1, TILE]),
                        )

                nc.sync.dma_start(
                    out=out[g * rpg : (g + 1) * rpg].rearrange("(o n) -> o n", o=1),
                    in_=stage[:],
                )
            if rep_cm is not None:
                rep_cm.__exit__(None, None, None)

    nc.compile()
    return nc


def prep_inputs(anchor_h, candidate_h, doc_embed, W1, b1, W2, b2,
                bc=BC, groups=GROUPS, cores=CORES):
    """Host-side packing of full inputs into per-core in_maps."""
    jpg = bc // groups // 128

    tbl16 = np.zeros((doc_embed.shape[0], DP), np.float16)
    tbl16[:, :EMBED] = np.asarray(doc_embed, np.float32)

    # W1 rows: [a(100); c(100); ac(100)] -> padded chunks of 128
    W1 = np.asarray(W1, np.float32)
    w1p = np.zeros((DP, 3 * H), np.float16)
    w1p[:EMBED, 0:H] = W1[0:EMBED]
    w1p[:EMBED, H : 2 * H] = W1[EMBED : 2 * EMBED]
    w1p[:EMBED, 2 * H : 3 * H] = W1[2 * EMBED : 3 * EMBED]

    w2p = np.asarray(W2, np.float32).astype(np.float16).reshape(H, 1)
    b1p = np.asarray(b1, np.float32).reshape(H, 1)
    b2p = np.asarray(b2, np.float32).reshape(1, 1)

    a_all = np.asarray(anchor_h).astype(np.int32)
    c_all = np.asarray(candidate_h).astype(np.int32)

    in_maps = []
    for c in range(cores):
        sl = slice(c * bc, (c + 1) * bc)
        # layout[p, g*jpg + j] = idx[g*rpg + j*128 + p]
        ia = (
            a_all[sl].reshape(groups, jpg, 128).transpose(2, 0, 1)
            .reshape(128, groups * jpg).copy()
        )
        icx = (
            c_all[sl].reshape(groups, jpg, 128).transpose(2, 0, 1)
            .reshape(128, groups * jpg).copy()
        )
        in_maps.append({
            "tbl": tbl16, "ia": ia, "ic": icx,
            "w1": w1p, "w2": w2p, "b1": b1p, "b2": b2p,
        })
    return in_maps


_NC_CACHE = {}


def get_nc():
    if "nc" not in _NC_CACHE:
        _NC_CACHE["nc"] = build_nc()
    return _NC_CACHE["nc"]


def kernel(anchor_h, candidate_h, doc_embed, W1, b1, W2, b2):
    nc = get_nc()
    in_maps = prep_inputs(anchor_h, candidate_h, doc_embed, W1, b1, W2, b2)
    res = run_bass_kernel_spmd(nc, in_maps, core_ids=list(range(CORES)))
    outs = [res.results[c]["out"] for c in range(CORES)]
    return np.concatenate(outs).reshape(BATCH, 1).astype(np.float32)
